# revision 9
# baseline (speedup 1.0000x reference)
"""Trainium2 Bass kernel: cross-entropy with Gaussian-smoothed labels.

loss = mean over tokens of  [ Wsum(t) * logsumexp(pred_row) - sum_k w_k * pred[start+k] ]

where the smoothed one-hot reduces exactly to a 7-tap window:
  start = clip(t-3, 0, C-7), u = t-start, w_k = f(k-u)
  f(0)=1.0, f(+-m)=exp(-2^m/4) for m in 1..3, else 0.

Sharding: pure data-parallel over the batch axis, 4 batches (8192 tokens)
per core across 8 cores. Per core:
  - stream pred [8192, 722] f32 through SBUF in [128, 4*722] tiles,
    ACT-engine Exp with accumulate -> per-token sum(exp); Ln -> lse.
  - 64 indirect DMAs gather the 7-wide windows (one offset per partition).
  - weights built once on-chip from iota/compares ([128, 64, 7]).
  - per-core partial sums [128, 1] DMA'd out; host sums 8x128 and divides.
"""
import math

import numpy as np

import concourse.bass as bass
import concourse.bacc as bacc
import concourse.tile as tile
from concourse import mybir
from concourse import bass_utils

B, T, C = 32, 2048, 722
CORES = 8
SHARD = B * T // CORES          # 8192 tokens per core
P = 128
TILES = SHARD // P              # 64
G = 4                           # token-tiles per DMA group
K = 7
START_MAX = C - K               # 715
DECAYS = [math.exp(-(2.0 ** d) / 4.0) for d in range(4)]

_ALU = mybir.AluOpType
_ACT = mybir.ActivationFunctionType

_NC = None


def _bcast_inner(ap, n):
    """Append a step-0 broadcast dim of length n to an AP."""
    return bass.AP(tensor=ap.tensor, offset=ap.offset, ap=[*ap.ap, [0, n]])


def _build(repeat=1, variant="full"):
    do_gather = variant in ("full", "noexp", "g32", "g64h", "gsep", "fullsep")
    do_dense = variant in ("full", "nogather", "fullsep")
    sep_gather = variant in ("gsep", "fullsep")
    nc = bacc.Bacc("TRN2", target_bir_lowering=False, debug=False,
                   enable_asserts=True, num_devices=CORES)
    pred = nc.dram_tensor("pred", [SHARD, C], mybir.dt.float32, kind="ExternalInput")
    target = nc.dram_tensor("target", [SHARD], mybir.dt.int32, kind="ExternalInput")
    out = nc.dram_tensor("partial", [P, 1], mybir.dt.float32, kind="ExternalOutput")

    pred_flat = pred.ap().rearrange("a b -> (a b)").rearrange("(n one) -> n one", one=1)
    # token index = jg*(G*P) + g*P + p
    pred_g = pred.ap().rearrange("(j g p) c -> j p g c", p=P, g=G)

    import contextlib
    with tile.TileContext(nc) as tc:
        with (tc.tile_pool(name="pred", bufs=3) as pred_pool,
              tc.tile_pool(name="exp", bufs=4) as exp_pool,
              tc.tile_pool(name="small", bufs=1) as small,
              (tc.For_i(0, repeat, 1) if repeat > 1 else contextlib.nullcontext())):
            # targets: tgt_sb[p, j] = target[j*128 + p]
            tgt_sb = small.tile([P, TILES], mybir.dt.int32)
            nc.sync.dma_start(out=tgt_sb, in_=target.ap().rearrange("(j p) -> p j", p=P))

            # flat element offsets of each token's window start
            row = small.tile([P, TILES], mybir.dt.int32)
            nc.gpsimd.iota(row, pattern=[[P, TILES]], base=0, channel_multiplier=1)
            start_i = small.tile([P, TILES], mybir.dt.int32)
            nc.vector.tensor_scalar(out=start_i, in0=tgt_sb, scalar1=3, scalar2=0,
                                    op0=_ALU.subtract, op1=_ALU.max)
            nc.vector.tensor_scalar_min(out=start_i, in0=start_i, scalar1=START_MAX)
            offs = small.tile([P, TILES], mybir.dt.int32)
            nc.vector.tensor_scalar_mul(out=offs, in0=row, scalar1=C)
            nc.vector.tensor_add(out=offs, in0=offs, in1=start_i)

            # u = t - start in f32 (0..6); diff[p,j,k] = k - u[p,j]
            tf = small.tile([P, TILES], mybir.dt.float32)
            nc.vector.tensor_copy(out=tf, in_=tgt_sb)
            sf = small.tile([P, TILES], mybir.dt.float32)
            nc.vector.tensor_copy(out=sf, in_=start_i)
            uf = small.tile([P, TILES], mybir.dt.float32)
            nc.vector.tensor_sub(out=uf, in0=tf, in1=sf)

            iok = small.tile([P, TILES, K], mybir.dt.float32)
            nc.gpsimd.iota(iok, pattern=[[0, TILES], [1, K]], base=0,
                           channel_multiplier=0, allow_small_or_imprecise_dtypes=True)
            diff = small.tile([P, TILES, K], mybir.dt.float32)
            nc.vector.scalar_tensor_tensor(out=diff, in0=iok, scalar=1.0,
                                           in1=_bcast_inner(uf, K),
                                           op0=_ALU.mult, op1=_ALU.subtract)
            # w = 1.0*(diff==0) + sum_m DECAYS[m]*(|diff|==m)
            w = small.tile([P, TILES, K], mybir.dt.float32)
            nc.vector.tensor_scalar(out=w, in0=diff, scalar1=0.0, scalar2=None,
                                    op0=_ALU.is_equal)
            tmp = small.tile([P, TILES, K], mybir.dt.float32)
            for m in (1, 2, 3):
                for s in (-m, m):
                    nc.vector.tensor_scalar(out=tmp, in0=diff, scalar1=float(s),
                                            scalar2=None, op0=_ALU.is_equal)
                    nc.vector.scalar_tensor_tensor(out=w, in0=tmp, scalar=DECAYS[m],
                                                   in1=w, op0=_ALU.mult, op1=_ALU.add)
            wsum = small.tile([P, TILES], mybir.dt.float32)
            nc.vector.reduce_sum(out=wsum, in_=w, axis=mybir.AxisListType.X)

            # windowed gathers: one indirect DMA per token-tile
            gath_tiles = None
            gath = small.tile([P, TILES, K], mybir.dt.float32)
            if do_gather and sep_gather:
                gath_tiles = []
                with tc.tile_pool(name="gath", bufs=TILES) as gpool:
                    for j in range(TILES):
                        gt = gpool.tile([P, K], mybir.dt.float32, tag="g")
                        nc.gpsimd.indirect_dma_start(
                            out=gt,
                            out_offset=None,
                            in_=pred_flat,
                            in_offset=bass.IndirectOffsetOnAxis(
                                ap=offs[:, j:j + 1], axis=0),
                        )
                        gath_tiles.append(gt)
            elif do_gather:
                ncalls, nparts = TILES, P
                if variant == "g32":
                    ncalls = 32
                elif variant == "g64h":
                    nparts = 64
                for j in range(ncalls):
                    nc.gpsimd.indirect_dma_start(
                        out=gath[:nparts, j, :],
                        out_offset=None,
                        in_=pred_flat,
                        in_offset=bass.IndirectOffsetOnAxis(
                            ap=offs[:nparts, j:j + 1], axis=0),
                    )
            else:
                nc.vector.memset(gath, 0.0)

            # dense stream: exp + accumulate -> sum(exp) per token
            sums = small.tile([P, TILES], mybir.dt.float32)
            if do_dense:
                for jg in range(TILES // G):
                    pt = pred_pool.tile([P, G, C], mybir.dt.float32)
                    nc.sync.dma_start(out=pt, in_=pred_g[jg])
                    for g in range(G):
                        et = exp_pool.tile([P, C], mybir.dt.float32)
                        j = jg * G + g
                        nc.scalar.activation(out=et, in_=pt[:, g, :], func=_ACT.Exp,
                                             accum_out=sums[:, j:j + 1])
            else:
                nc.vector.memset(sums, 1.0)

            # lse, weighted gather sums, per-core partial
            lse = small.tile([P, TILES], mybir.dt.float32)
            nc.scalar.activation(out=lse, in_=sums, func=_ACT.Ln)
            wg = small.tile([P, TILES, K], mybir.dt.float32)
            gsum = small.tile([P, TILES], mybir.dt.float32)
            if gath_tiles is not None:
                for j in range(TILES):
                    nc.vector.affine_mul_reduce(
                        out=wg[:, j, :], accum_out=gsum[:, j:j + 1],
                        in0=w[:, j, :], in1=gath_tiles[j], scale=1.0, bias=0.0)
            else:
                nc.vector.tensor_mul(out=wg, in0=w, in1=gath)
                nc.vector.reduce_sum(out=gsum, in_=wg, axis=mybir.AxisListType.X)
            loss = small.tile([P, TILES], mybir.dt.float32)
            nc.vector.tensor_mul(out=loss, in0=wsum, in1=lse)
            nc.vector.tensor_sub(out=loss, in0=loss, in1=gsum)
            part = small.tile([P, 1], mybir.dt.float32)
            nc.vector.reduce_sum(out=part, in_=loss, axis=mybir.AxisListType.X)
            nc.sync.dma_start(out=out.ap(), in_=part)
    nc.compile()
    return nc


def _get_nc():
    global _NC
    if _NC is None:
        _NC = _build()
    return _NC


def _shard_inputs(pred, target):
    bpc = B // CORES
    in_maps = []
    for c in range(CORES):
        in_maps.append({
            "pred": np.ascontiguousarray(
                pred[c * bpc:(c + 1) * bpc].reshape(SHARD, C), dtype=np.float32),
            "target": np.ascontiguousarray(
                target[c * bpc:(c + 1) * bpc].reshape(SHARD), dtype=np.int32),
        })
    return in_maps


def _run(pred, target, **kwargs):
    nc = _get_nc()
    return bass_utils.run_bass_kernel_spmd(
        nc, _shard_inputs(pred, target), core_ids=list(range(CORES)), **kwargs)


def kernel(pred, target):
    res = _run(pred, target)
    total = sum(float(r["partial"].astype(np.float64).sum()) for r in res.results)
    return np.asarray(total / (B * T), dtype=np.float32)


# revision 11
# speedup vs baseline: 1.0195x; 1.0195x over previous
"""Trainium2 Bass kernel: cross-entropy with Gaussian-smoothed labels.

loss = mean over tokens of  [ Wsum(t) * logsumexp(pred_row) - sum_k w_k * pred[start+k] ]

where the smoothed one-hot reduces exactly to a 7-tap window:
  start = clip(t-3, 0, C-7), u = t-start, w_k = f(k-u)
  f(0)=1.0, f(+-m)=exp(-2^m/4) for m in 1..3, else 0.

Sharding: pure data-parallel over the batch axis, 4 batches (8192 tokens)
per core across 8 cores. Per core:
  - stream pred [8192, 722] f32 through SBUF in [128, 4*722] tiles,
    ACT-engine Exp with accumulate -> per-token sum(exp); Ln -> lse.
  - 64 indirect DMAs gather the 7-wide windows (one offset per partition).
  - weights built once on-chip from iota/compares ([128, 64, 7]).
  - per-core partial sums [128, 1] DMA'd out; host sums 8x128 and divides.
"""
import math

import numpy as np

import concourse.bass as bass
import concourse.bacc as bacc
import concourse.tile as tile
from concourse import mybir
from concourse import bass_utils

B, T, C = 32, 2048, 722
CORES = 8
SHARD = B * T // CORES          # 8192 tokens per core
P = 128
TILES = SHARD // P              # 64
G = 4                           # token-tiles per DMA group
K = 7
START_MAX = C - K               # 715
DECAYS = [math.exp(-(2.0 ** d) / 4.0) for d in range(4)]

_ALU = mybir.AluOpType
_ACT = mybir.ActivationFunctionType

_NC = None


def _bcast_inner(ap, n):
    """Append a step-0 broadcast dim of length n to an AP."""
    return bass.AP(tensor=ap.tensor, offset=ap.offset, ap=[*ap.ap, [0, n]])


def _build(repeat=1, variant="full", G=4, two_ring=False, dve_tiles=0):
    do_gather = variant in ("full", "noexp", "g32", "g64h", "gsep", "fullsep")
    do_dense = variant in ("full", "nogather", "fullsep")
    sep_gather = variant in ("gsep", "fullsep")
    nc = bacc.Bacc("TRN2", target_bir_lowering=False, debug=False,
                   enable_asserts=True, num_devices=CORES)
    pred = nc.dram_tensor("pred", [SHARD, C], mybir.dt.float32, kind="ExternalInput")
    target = nc.dram_tensor("target", [SHARD], mybir.dt.int32, kind="ExternalInput")
    out = nc.dram_tensor("partial", [P, 1], mybir.dt.float32, kind="ExternalOutput")

    pred_flat = pred.ap().rearrange("a b -> (a b)").rearrange("(n one) -> n one", one=1)
    # token index = jg*(G*P) + g*P + p
    pred_g = pred.ap().rearrange("(j g p) c -> j p g c", p=P, g=G)
    # token-tiles whose gather runs on DVE, spread evenly over the stream
    dve_set = set() if dve_tiles == 0 else \
        set(int(round(i * TILES / dve_tiles)) for i in range(dve_tiles))

    import contextlib
    with tile.TileContext(nc) as tc:
        with (tc.tile_pool(name="pred", bufs=3) as pred_pool,
              tc.tile_pool(name="exp", bufs=4) as exp_pool,
              tc.tile_pool(name="small", bufs=1) as small,
              (tc.For_i(0, repeat, 1) if repeat > 1 else contextlib.nullcontext())):
            # targets: tgt_sb[p, j] = target[j*128 + p]
            tgt_sb = small.tile([P, TILES], mybir.dt.int32)
            nc.sync.dma_start(out=tgt_sb, in_=target.ap().rearrange("(j p) -> p j", p=P))

            # flat element offsets of each token's window start
            row = small.tile([P, TILES], mybir.dt.int32)
            nc.gpsimd.iota(row, pattern=[[P, TILES]], base=0, channel_multiplier=1)
            start_i = small.tile([P, TILES], mybir.dt.int32)
            nc.vector.tensor_scalar(out=start_i, in0=tgt_sb, scalar1=3, scalar2=0,
                                    op0=_ALU.subtract, op1=_ALU.max)
            nc.vector.tensor_scalar_min(out=start_i, in0=start_i, scalar1=START_MAX)
            offs = small.tile([P, TILES], mybir.dt.int32)
            nc.vector.tensor_scalar_mul(out=offs, in0=row, scalar1=C)
            nc.vector.tensor_add(out=offs, in0=offs, in1=start_i)

            # u = t - start in f32 (0..6); diff[p,j,k] = k - u[p,j]
            tf = small.tile([P, TILES], mybir.dt.float32)
            nc.vector.tensor_copy(out=tf, in_=tgt_sb)
            sf = small.tile([P, TILES], mybir.dt.float32)
            nc.vector.tensor_copy(out=sf, in_=start_i)
            uf = small.tile([P, TILES], mybir.dt.float32)
            nc.vector.tensor_sub(out=uf, in0=tf, in1=sf)

            iok = small.tile([P, TILES, K], mybir.dt.float32)
            nc.gpsimd.iota(iok, pattern=[[0, TILES], [1, K]], base=0,
                           channel_multiplier=0, allow_small_or_imprecise_dtypes=True)
            diff = small.tile([P, TILES, K], mybir.dt.float32)
            nc.vector.scalar_tensor_tensor(out=diff, in0=iok, scalar=1.0,
                                           in1=_bcast_inner(uf, K),
                                           op0=_ALU.mult, op1=_ALU.subtract)
            # w = 1.0*(diff==0) + sum_m DECAYS[m]*(|diff|==m)
            w = small.tile([P, TILES, K], mybir.dt.float32)
            nc.vector.tensor_scalar(out=w, in0=diff, scalar1=0.0, scalar2=None,
                                    op0=_ALU.is_equal)
            tmp = small.tile([P, TILES, K], mybir.dt.float32)
            for m in (1, 2, 3):
                for s in (-m, m):
                    nc.vector.tensor_scalar(out=tmp, in0=diff, scalar1=float(s),
                                            scalar2=None, op0=_ALU.is_equal)
                    nc.vector.scalar_tensor_tensor(out=w, in0=tmp, scalar=DECAYS[m],
                                                   in1=w, op0=_ALU.mult, op1=_ALU.add)
            wsum = small.tile([P, TILES], mybir.dt.float32)
            nc.vector.reduce_sum(out=wsum, in_=w, axis=mybir.AxisListType.X)

            # windowed gathers: one indirect DMA per token-tile
            gath_tiles = None
            gath = small.tile([P, TILES, K], mybir.dt.float32)
            if do_gather and sep_gather:
                gath_tiles = []
                with tc.tile_pool(name="gath", bufs=TILES) as gpool:
                    for j in range(TILES):
                        gt = gpool.tile([P, K], mybir.dt.float32, tag="g")
                        nc.gpsimd.indirect_dma_start(
                            out=gt,
                            out_offset=None,
                            in_=pred_flat,
                            in_offset=bass.IndirectOffsetOnAxis(
                                ap=offs[:, j:j + 1], axis=0),
                        )
                        gath_tiles.append(gt)
            elif do_gather:
                ncalls, nparts = TILES, P
                if variant == "g32":
                    ncalls = 32
                elif variant == "g64h":
                    nparts = 64
                for j in range(ncalls):
                    if j in dve_set:
                        continue
                    nc.gpsimd.indirect_dma_start(
                        out=gath[:nparts, j, :],
                        out_offset=None,
                        in_=pred_flat,
                        in_offset=bass.IndirectOffsetOnAxis(
                            ap=offs[:nparts, j:j + 1], axis=0),
                    )
                for j in sorted(dve_set):
                    nc.vector.memset(gath[:, j, :], 0.0)
            else:
                nc.vector.memset(gath, 0.0)

            # dense stream: exp + accumulate -> sum(exp) per token
            sums = small.tile([P, TILES], mybir.dt.float32)
            gsum_d = small.tile([P, TILES], mybir.dt.float32)
            if dve_set:
                nc.vector.memset(gsum_d, 0.0)
                iota722 = small.tile([P, C], mybir.dt.float32)
                nc.gpsimd.iota(iota722, pattern=[[1, C]], base=0,
                               channel_multiplier=0,
                               allow_small_or_imprecise_dtypes=True)
                wdense = small.tile([P, C], mybir.dt.float32)
                eqd = small.tile([P, C], mybir.dt.float32)
                diffd = small.tile([P, C], mybir.dt.float32)
                wgd = small.tile([P, C], mybir.dt.float32)
            if do_dense:
                for jg in range(TILES // G):
                    pt = pred_pool.tile([P, G, C], mybir.dt.float32)
                    dma_eng = nc.scalar if (two_ring and jg % 2) else nc.sync
                    dma_eng.dma_start(out=pt, in_=pred_g[jg])
                    for g in range(G):
                        et = exp_pool.tile([P, C], mybir.dt.float32)
                        j = jg * G + g
                        nc.scalar.activation(out=et, in_=pt[:, g, :], func=_ACT.Exp,
                                             accum_out=sums[:, j:j + 1])
                        if j in dve_set:
                            # dense masked weighted sum on DVE for this tile
                            nc.vector.tensor_scalar(
                                out=diffd, in0=iota722, scalar1=tf[:, j:j + 1],
                                scalar2=None, op0=_ALU.subtract)
                            nc.vector.tensor_scalar(
                                out=wdense, in0=diffd, scalar1=0.0, scalar2=None,
                                op0=_ALU.is_equal)
                            for m in (1, 2, 3):
                                for s in (-m, m):
                                    nc.vector.tensor_scalar(
                                        out=eqd, in0=diffd, scalar1=float(s),
                                        scalar2=None, op0=_ALU.is_equal)
                                    nc.vector.scalar_tensor_tensor(
                                        out=wdense, in0=eqd, scalar=DECAYS[m],
                                        in1=wdense, op0=_ALU.mult, op1=_ALU.add)
                            nc.vector.affine_mul_reduce(
                                out=wgd, accum_out=gsum_d[:, j:j + 1],
                                in0=wdense, in1=pt[:, g, :], scale=1.0, bias=0.0)
            else:
                nc.vector.memset(sums, 1.0)

            # lse, weighted gather sums, per-core partial
            lse = small.tile([P, TILES], mybir.dt.float32)
            nc.scalar.activation(out=lse, in_=sums, func=_ACT.Ln)
            wg = small.tile([P, TILES, K], mybir.dt.float32)
            gsum = small.tile([P, TILES], mybir.dt.float32)
            if gath_tiles is not None:
                for j in range(TILES):
                    nc.vector.affine_mul_reduce(
                        out=wg[:, j, :], accum_out=gsum[:, j:j + 1],
                        in0=w[:, j, :], in1=gath_tiles[j], scale=1.0, bias=0.0)
            else:
                nc.vector.tensor_mul(out=wg, in0=w, in1=gath)
                nc.vector.reduce_sum(out=gsum, in_=wg, axis=mybir.AxisListType.X)
            if dve_set:
                nc.vector.tensor_add(out=gsum, in0=gsum, in1=gsum_d)
            loss = small.tile([P, TILES], mybir.dt.float32)
            nc.vector.tensor_mul(out=loss, in0=wsum, in1=lse)
            nc.vector.tensor_sub(out=loss, in0=loss, in1=gsum)
            part = small.tile([P, 1], mybir.dt.float32)
            nc.vector.reduce_sum(out=part, in_=loss, axis=mybir.AxisListType.X)
            nc.sync.dma_start(out=out.ap(), in_=part)
    nc.compile()
    return nc


def _get_nc():
    global _NC
    if _NC is None:
        _NC = _build()
    return _NC


def _shard_inputs(pred, target):
    bpc = B // CORES
    in_maps = []
    for c in range(CORES):
        in_maps.append({
            "pred": np.ascontiguousarray(
                pred[c * bpc:(c + 1) * bpc].reshape(SHARD, C), dtype=np.float32),
            "target": np.ascontiguousarray(
                target[c * bpc:(c + 1) * bpc].reshape(SHARD), dtype=np.int32),
        })
    return in_maps


def _run(pred, target, **kwargs):
    nc = _get_nc()
    return bass_utils.run_bass_kernel_spmd(
        nc, _shard_inputs(pred, target), core_ids=list(range(CORES)), **kwargs)


def kernel(pred, target):
    res = _run(pred, target)
    total = sum(float(r["partial"].astype(np.float64).sum()) for r in res.results)
    return np.asarray(total / (B * T), dtype=np.float32)


# revision 12
# speedup vs baseline: 1.1900x; 1.1672x over previous
"""Trainium2 Bass kernel: cross-entropy with Gaussian-smoothed labels.

loss = mean over tokens of  [ Wsum(t) * logsumexp(pred_row) - sum_k w_k * pred[start+k] ]

where the smoothed one-hot reduces exactly to a 7-tap window:
  start = clip(t-3, 0, C-7), u = t-start, w_k = f(k-u)
  f(0)=1.0, f(+-m)=exp(-2^m/4) for m in 1..3, else 0.

Sharding: pure data-parallel over the batch axis, 4 batches (8192 tokens)
per core across 8 cores. Per core:
  - stream pred [8192, 722] f32 through SBUF in [128, 4*722] tiles,
    ACT-engine Exp with accumulate -> per-token sum(exp); Ln -> lse.
  - 64 indirect DMAs gather the 7-wide windows (one offset per partition).
  - weights built once on-chip from iota/compares ([128, 64, 7]).
  - per-core partial sums [128, 1] DMA'd out; host sums 8x128 and divides.
"""
import math

import numpy as np

import concourse.bass as bass
import concourse.bacc as bacc
import concourse.tile as tile
from concourse import mybir
from concourse import bass_utils

B, T, C = 32, 2048, 722
CORES = 8
SHARD = B * T // CORES          # 8192 tokens per core
P = 128
TILES = SHARD // P              # 64
G = 4                           # token-tiles per DMA group
K = 7
START_MAX = C - K               # 715
DECAYS = [math.exp(-(2.0 ** d) / 4.0) for d in range(4)]

_ALU = mybir.AluOpType
_ACT = mybir.ActivationFunctionType

_NC = None


def _bcast_inner(ap, n):
    """Append a step-0 broadcast dim of length n to an AP."""
    return bass.AP(tensor=ap.tensor, offset=ap.offset, ap=[*ap.ap, [0, n]])


def _build(repeat=1, variant="full", G=4, two_ring=False, dve_tiles=0):
    do_gather = variant in ("full", "noexp", "g32", "g64h", "gsep", "fullsep")
    do_dense = variant in ("full", "nogather", "fullsep")
    sep_gather = variant in ("gsep", "fullsep")
    nc = bacc.Bacc("TRN2", target_bir_lowering=False, debug=False,
                   enable_asserts=True, num_devices=CORES)
    pred = nc.dram_tensor("pred", [SHARD, C], mybir.dt.float32, kind="ExternalInput")
    target = nc.dram_tensor("target", [SHARD], mybir.dt.int32, kind="ExternalInput")
    out = nc.dram_tensor("partial", [P, 1], mybir.dt.float32, kind="ExternalOutput")

    pred_flat = pred.ap().rearrange("a b -> (a b)").rearrange("(n one) -> n one", one=1)
    # token index = p*TILES + jg*G + g  (each partition owns a contiguous slab)
    pred_g = pred.ap().rearrange("(p j g) c -> j p g c", p=P, g=G)
    # token-tiles whose gather runs on DVE, spread evenly over the stream
    dve_set = set() if dve_tiles == 0 else \
        set(int(round(i * TILES / dve_tiles)) for i in range(dve_tiles))

    import contextlib
    with tile.TileContext(nc) as tc:
        with (tc.tile_pool(name="pred", bufs=3) as pred_pool,
              tc.tile_pool(name="exp", bufs=4) as exp_pool,
              tc.tile_pool(name="small", bufs=1) as small,
              (tc.For_i(0, repeat, 1) if repeat > 1 else contextlib.nullcontext())):
            # targets: tgt_sb[p, j] = target[j*128 + p]
            tgt_sb = small.tile([P, TILES], mybir.dt.int32)
            nc.sync.dma_start(out=tgt_sb, in_=target.ap().rearrange("(p j) -> p j", p=P))

            # flat element offsets of each token's window start
            row = small.tile([P, TILES], mybir.dt.int32)
            nc.gpsimd.iota(row, pattern=[[1, TILES]], base=0, channel_multiplier=TILES)
            start_i = small.tile([P, TILES], mybir.dt.int32)
            nc.vector.tensor_scalar(out=start_i, in0=tgt_sb, scalar1=3, scalar2=0,
                                    op0=_ALU.subtract, op1=_ALU.max)
            nc.vector.tensor_scalar_min(out=start_i, in0=start_i, scalar1=START_MAX)
            offs = small.tile([P, TILES], mybir.dt.int32)
            nc.vector.tensor_scalar_mul(out=offs, in0=row, scalar1=C)
            nc.vector.tensor_add(out=offs, in0=offs, in1=start_i)

            # u = t - start in f32 (0..6); diff[p,j,k] = k - u[p,j]
            tf = small.tile([P, TILES], mybir.dt.float32)
            nc.vector.tensor_copy(out=tf, in_=tgt_sb)
            sf = small.tile([P, TILES], mybir.dt.float32)
            nc.vector.tensor_copy(out=sf, in_=start_i)
            uf = small.tile([P, TILES], mybir.dt.float32)
            nc.vector.tensor_sub(out=uf, in0=tf, in1=sf)

            iok = small.tile([P, TILES, K], mybir.dt.float32)
            nc.gpsimd.iota(iok, pattern=[[0, TILES], [1, K]], base=0,
                           channel_multiplier=0, allow_small_or_imprecise_dtypes=True)
            diff = small.tile([P, TILES, K], mybir.dt.float32)
            nc.vector.scalar_tensor_tensor(out=diff, in0=iok, scalar=1.0,
                                           in1=_bcast_inner(uf, K),
                                           op0=_ALU.mult, op1=_ALU.subtract)
            # w = 1.0*(diff==0) + sum_m DECAYS[m]*(|diff|==m)
            w = small.tile([P, TILES, K], mybir.dt.float32)
            nc.vector.tensor_scalar(out=w, in0=diff, scalar1=0.0, scalar2=None,
                                    op0=_ALU.is_equal)
            tmp = small.tile([P, TILES, K], mybir.dt.float32)
            for m in (1, 2, 3):
                for s in (-m, m):
                    nc.vector.tensor_scalar(out=tmp, in0=diff, scalar1=float(s),
                                            scalar2=None, op0=_ALU.is_equal)
                    nc.vector.scalar_tensor_tensor(out=w, in0=tmp, scalar=DECAYS[m],
                                                   in1=w, op0=_ALU.mult, op1=_ALU.add)
            wsum = small.tile([P, TILES], mybir.dt.float32)
            nc.vector.reduce_sum(out=wsum, in_=w, axis=mybir.AxisListType.X)

            # windowed gathers: one indirect DMA per token-tile
            gath_tiles = None
            gath = small.tile([P, TILES, K], mybir.dt.float32)
            if do_gather and sep_gather:
                gath_tiles = []
                with tc.tile_pool(name="gath", bufs=TILES) as gpool:
                    for j in range(TILES):
                        gt = gpool.tile([P, K], mybir.dt.float32, tag="g")
                        nc.gpsimd.indirect_dma_start(
                            out=gt,
                            out_offset=None,
                            in_=pred_flat,
                            in_offset=bass.IndirectOffsetOnAxis(
                                ap=offs[:, j:j + 1], axis=0),
                        )
                        gath_tiles.append(gt)
            elif do_gather:
                ncalls, nparts = TILES, P
                if variant == "g32":
                    ncalls = 32
                elif variant == "g64h":
                    nparts = 64
                for j in range(ncalls):
                    if j in dve_set:
                        continue
                    nc.gpsimd.indirect_dma_start(
                        out=gath[:nparts, j, :],
                        out_offset=None,
                        in_=pred_flat,
                        in_offset=bass.IndirectOffsetOnAxis(
                            ap=offs[:nparts, j:j + 1], axis=0),
                    )
                for j in sorted(dve_set):
                    nc.vector.memset(gath[:, j, :], 0.0)
            else:
                nc.vector.memset(gath, 0.0)

            # dense stream: exp + accumulate -> sum(exp) per token
            sums = small.tile([P, TILES], mybir.dt.float32)
            gsum_d = small.tile([P, TILES], mybir.dt.float32)
            if dve_set:
                nc.vector.memset(gsum_d, 0.0)
                iota722 = small.tile([P, C], mybir.dt.float32)
                nc.gpsimd.iota(iota722, pattern=[[1, C]], base=0,
                               channel_multiplier=0,
                               allow_small_or_imprecise_dtypes=True)
                wdense = small.tile([P, C], mybir.dt.float32)
                eqd = small.tile([P, C], mybir.dt.float32)
                diffd = small.tile([P, C], mybir.dt.float32)
                wgd = small.tile([P, C], mybir.dt.float32)
            if do_dense:
                for jg in range(TILES // G):
                    pt = pred_pool.tile([P, G, C], mybir.dt.float32)
                    dma_eng = nc.scalar if (two_ring and jg % 2) else nc.sync
                    dma_eng.dma_start(out=pt, in_=pred_g[jg])
                    for g in range(G):
                        et = exp_pool.tile([P, C], mybir.dt.float32)
                        j = jg * G + g
                        nc.scalar.activation(out=et, in_=pt[:, g, :], func=_ACT.Exp,
                                             accum_out=sums[:, j:j + 1])
                        if j in dve_set:
                            # dense masked weighted sum on DVE for this tile
                            nc.vector.tensor_scalar(
                                out=diffd, in0=iota722, scalar1=tf[:, j:j + 1],
                                scalar2=None, op0=_ALU.subtract)
                            nc.vector.tensor_scalar(
                                out=wdense, in0=diffd, scalar1=0.0, scalar2=None,
                                op0=_ALU.is_equal)
                            for m in (1, 2, 3):
                                for s in (-m, m):
                                    nc.vector.tensor_scalar(
                                        out=eqd, in0=diffd, scalar1=float(s),
                                        scalar2=None, op0=_ALU.is_equal)
                                    nc.vector.scalar_tensor_tensor(
                                        out=wdense, in0=eqd, scalar=DECAYS[m],
                                        in1=wdense, op0=_ALU.mult, op1=_ALU.add)
                            nc.vector.affine_mul_reduce(
                                out=wgd, accum_out=gsum_d[:, j:j + 1],
                                in0=wdense, in1=pt[:, g, :], scale=1.0, bias=0.0)
            else:
                nc.vector.memset(sums, 1.0)

            # lse, weighted gather sums, per-core partial
            lse = small.tile([P, TILES], mybir.dt.float32)
            nc.scalar.activation(out=lse, in_=sums, func=_ACT.Ln)
            wg = small.tile([P, TILES, K], mybir.dt.float32)
            gsum = small.tile([P, TILES], mybir.dt.float32)
            if gath_tiles is not None:
                for j in range(TILES):
                    nc.vector.affine_mul_reduce(
                        out=wg[:, j, :], accum_out=gsum[:, j:j + 1],
                        in0=w[:, j, :], in1=gath_tiles[j], scale=1.0, bias=0.0)
            else:
                nc.vector.tensor_mul(out=wg, in0=w, in1=gath)
                nc.vector.reduce_sum(out=gsum, in_=wg, axis=mybir.AxisListType.X)
            if dve_set:
                nc.vector.tensor_add(out=gsum, in0=gsum, in1=gsum_d)
            loss = small.tile([P, TILES], mybir.dt.float32)
            nc.vector.tensor_mul(out=loss, in0=wsum, in1=lse)
            nc.vector.tensor_sub(out=loss, in0=loss, in1=gsum)
            part = small.tile([P, 1], mybir.dt.float32)
            nc.vector.reduce_sum(out=part, in_=loss, axis=mybir.AxisListType.X)
            nc.sync.dma_start(out=out.ap(), in_=part)
    nc.compile()
    return nc


def _get_nc():
    global _NC
    if _NC is None:
        _NC = _build()
    return _NC


def _shard_inputs(pred, target):
    bpc = B // CORES
    in_maps = []
    for c in range(CORES):
        in_maps.append({
            "pred": np.ascontiguousarray(
                pred[c * bpc:(c + 1) * bpc].reshape(SHARD, C), dtype=np.float32),
            "target": np.ascontiguousarray(
                target[c * bpc:(c + 1) * bpc].reshape(SHARD), dtype=np.int32),
        })
    return in_maps


def _run(pred, target, **kwargs):
    nc = _get_nc()
    return bass_utils.run_bass_kernel_spmd(
        nc, _shard_inputs(pred, target), core_ids=list(range(CORES)), **kwargs)


def kernel(pred, target):
    res = _run(pred, target)
    total = sum(float(r["partial"].astype(np.float64).sum()) for r in res.results)
    return np.asarray(total / (B * T), dtype=np.float32)


# revision 13
# speedup vs baseline: 2.5216x; 2.1190x over previous
"""Trainium2 Bass kernel: cross-entropy with Gaussian-smoothed labels.

loss = mean over tokens of  [ Wsum(t) * logsumexp(pred_row) - sum_k w_k * pred[start+k] ]

where the smoothed one-hot reduces exactly to a 7-tap window:
  start = clip(t-3, 0, C-7), u = t-start, w_k = f(k-u)
  f(0)=1.0, f(+-m)=exp(-2^m/4) for m in 1..3, else 0.

Sharding: pure data-parallel over the batch axis, 4 batches (8192 tokens)
per core across 8 cores. Per core:
  - stream pred [8192, 722] f32 through SBUF in [128, 4*722] tiles,
    ACT-engine Exp with accumulate -> per-token sum(exp); Ln -> lse.
  - 64 indirect DMAs gather the 7-wide windows (one offset per partition).
  - weights built once on-chip from iota/compares ([128, 64, 7]).
  - per-core partial sums [128, 1] DMA'd out; host sums 8x128 and divides.
"""
import math

import numpy as np

import concourse.bass as bass
import concourse.bacc as bacc
import concourse.tile as tile
from concourse import mybir
from concourse import bass_utils

B, T, C = 32, 2048, 722
CORES = 8
SHARD = B * T // CORES          # 8192 tokens per core
P = 128
TILES = SHARD // P              # 64
G = 4                           # token-tiles per DMA group
K = 7
START_MAX = C - K               # 715
DECAYS = [math.exp(-(2.0 ** d) / 4.0) for d in range(4)]

_ALU = mybir.AluOpType
_ACT = mybir.ActivationFunctionType

_NC = None


def _bcast_inner(ap, n):
    """Append a step-0 broadcast dim of length n to an AP."""
    return bass.AP(tensor=ap.tensor, offset=ap.offset, ap=[*ap.ap, [0, n]])


def _build(repeat=1, variant="full", G=4, two_ring=False, dve_tiles=0):
    do_gather = variant in ("full", "noexp", "g32", "g64h", "gsep", "fullsep")
    do_dense = variant in ("full", "nogather", "fullsep", "dmaonly")
    act_stride = len_g = 1
    if variant == "dmaonly":
        act_stride = 8
    sep_gather = variant in ("gsep", "fullsep")
    nc = bacc.Bacc("TRN2", target_bir_lowering=False, debug=False,
                   enable_asserts=True, num_devices=CORES)
    pred = nc.dram_tensor("pred", [SHARD, C], mybir.dt.float32, kind="ExternalInput")
    target = nc.dram_tensor("target", [SHARD], mybir.dt.int32, kind="ExternalInput")
    out = nc.dram_tensor("partial", [P, 1], mybir.dt.float32, kind="ExternalOutput")

    pred_flat = pred.ap().rearrange("a b -> (a b)").rearrange("(n one) -> n one", one=1)
    # token index = p*TILES + jg*G + g  (each partition owns a contiguous slab)
    pred_g = pred.ap().rearrange("(p j g) c -> j p g c", p=P, g=G)
    # token-tiles whose gather runs on DVE, spread evenly over the stream
    dve_set = set() if dve_tiles == 0 else \
        set(int(round(i * TILES / dve_tiles)) for i in range(dve_tiles))

    import contextlib
    with tile.TileContext(nc) as tc:
        with (tc.tile_pool(name="pred", bufs=3) as pred_pool,
              tc.tile_pool(name="exp", bufs=4) as exp_pool,
              tc.tile_pool(name="small", bufs=1) as small,
              (tc.For_i(0, repeat, 1) if repeat > 1 else contextlib.nullcontext())):
            # targets: tgt_sb[p, j] = target[j*128 + p]
            tgt_sb = small.tile([P, TILES], mybir.dt.int32)
            nc.sync.dma_start(out=tgt_sb, in_=target.ap().rearrange("(p j) -> p j", p=P))

            # flat element offsets of each token's window start
            row = small.tile([P, TILES], mybir.dt.int32)
            nc.gpsimd.iota(row, pattern=[[1, TILES]], base=0, channel_multiplier=TILES)
            start_i = small.tile([P, TILES], mybir.dt.int32)
            nc.vector.tensor_scalar(out=start_i, in0=tgt_sb, scalar1=3, scalar2=0,
                                    op0=_ALU.subtract, op1=_ALU.max)
            nc.vector.tensor_scalar_min(out=start_i, in0=start_i, scalar1=START_MAX)
            offs = small.tile([P, TILES], mybir.dt.int32)
            nc.vector.tensor_scalar_mul(out=offs, in0=row, scalar1=C)
            nc.vector.tensor_add(out=offs, in0=offs, in1=start_i)

            # u = t - start in f32 (0..6); diff[p,j,k] = k - u[p,j]
            tf = small.tile([P, TILES], mybir.dt.float32)
            nc.vector.tensor_copy(out=tf, in_=tgt_sb)
            sf = small.tile([P, TILES], mybir.dt.float32)
            nc.vector.tensor_copy(out=sf, in_=start_i)
            uf = small.tile([P, TILES], mybir.dt.float32)
            nc.vector.tensor_sub(out=uf, in0=tf, in1=sf)

            iok = small.tile([P, TILES, K], mybir.dt.float32)
            nc.gpsimd.iota(iok, pattern=[[0, TILES], [1, K]], base=0,
                           channel_multiplier=0, allow_small_or_imprecise_dtypes=True)
            diff = small.tile([P, TILES, K], mybir.dt.float32)
            nc.vector.scalar_tensor_tensor(out=diff, in0=iok, scalar=1.0,
                                           in1=_bcast_inner(uf, K),
                                           op0=_ALU.mult, op1=_ALU.subtract)
            # w = 1.0*(diff==0) + sum_m DECAYS[m]*(|diff|==m)
            w = small.tile([P, TILES, K], mybir.dt.float32)
            nc.vector.tensor_scalar(out=w, in0=diff, scalar1=0.0, scalar2=None,
                                    op0=_ALU.is_equal)
            tmp = small.tile([P, TILES, K], mybir.dt.float32)
            for m in (1, 2, 3):
                for s in (-m, m):
                    nc.vector.tensor_scalar(out=tmp, in0=diff, scalar1=float(s),
                                            scalar2=None, op0=_ALU.is_equal)
                    nc.vector.scalar_tensor_tensor(out=w, in0=tmp, scalar=DECAYS[m],
                                                   in1=w, op0=_ALU.mult, op1=_ALU.add)
            wsum = small.tile([P, TILES], mybir.dt.float32)
            nc.vector.reduce_sum(out=wsum, in_=w, axis=mybir.AxisListType.X)

            # windowed gathers: one indirect DMA per token-tile
            gath_tiles = None
            gath = small.tile([P, TILES, K], mybir.dt.float32)
            if do_gather and sep_gather:
                gath_tiles = []
                with tc.tile_pool(name="gath", bufs=TILES) as gpool:
                    for j in range(TILES):
                        gt = gpool.tile([P, K], mybir.dt.float32, tag="g")
                        nc.gpsimd.indirect_dma_start(
                            out=gt,
                            out_offset=None,
                            in_=pred_flat,
                            in_offset=bass.IndirectOffsetOnAxis(
                                ap=offs[:, j:j + 1], axis=0),
                        )
                        gath_tiles.append(gt)
            elif do_gather:
                ncalls, nparts = TILES, P
                if variant == "g32":
                    ncalls = 32
                elif variant == "g64h":
                    nparts = 64
                for j in range(ncalls):
                    if j in dve_set:
                        continue
                    nc.gpsimd.indirect_dma_start(
                        out=gath[:nparts, j, :],
                        out_offset=None,
                        in_=pred_flat,
                        in_offset=bass.IndirectOffsetOnAxis(
                            ap=offs[:nparts, j:j + 1], axis=0),
                    )
                for j in sorted(dve_set):
                    nc.vector.memset(gath[:, j, :], 0.0)
            else:
                nc.vector.memset(gath, 0.0)

            # dense stream: exp + accumulate -> sum(exp) per token
            sums = small.tile([P, TILES], mybir.dt.float32)
            gsum_d = small.tile([P, TILES], mybir.dt.float32)
            if dve_set:
                nc.vector.memset(gsum_d, 0.0)
                iota722 = small.tile([P, C], mybir.dt.float32)
                nc.gpsimd.iota(iota722, pattern=[[1, C]], base=0,
                               channel_multiplier=0,
                               allow_small_or_imprecise_dtypes=True)
                wdense = small.tile([P, C], mybir.dt.float32)
                eqd = small.tile([P, C], mybir.dt.float32)
                diffd = small.tile([P, C], mybir.dt.float32)
                wgd = small.tile([P, C], mybir.dt.float32)
            if do_dense:
                for jg in range(TILES // G):
                    pt = pred_pool.tile([P, G, C], mybir.dt.float32)
                    dma_eng = nc.scalar if (two_ring and jg % 2) else nc.sync
                    dma_eng.dma_start(out=pt, in_=pred_g[jg])
                    for g in range(G):
                        j = jg * G + g
                        if j % act_stride:
                            continue
                        et = exp_pool.tile([P, C], mybir.dt.float32)
                        nc.scalar.activation(out=et, in_=pt[:, g, :], func=_ACT.Exp,
                                             accum_out=sums[:, j:j + 1])
                        if j in dve_set:
                            # dense masked weighted sum on DVE for this tile
                            nc.vector.tensor_scalar(
                                out=diffd, in0=iota722, scalar1=tf[:, j:j + 1],
                                scalar2=None, op0=_ALU.subtract)
                            nc.vector.tensor_scalar(
                                out=wdense, in0=diffd, scalar1=0.0, scalar2=None,
                                op0=_ALU.is_equal)
                            for m in (1, 2, 3):
                                for s in (-m, m):
                                    nc.vector.tensor_scalar(
                                        out=eqd, in0=diffd, scalar1=float(s),
                                        scalar2=None, op0=_ALU.is_equal)
                                    nc.vector.scalar_tensor_tensor(
                                        out=wdense, in0=eqd, scalar=DECAYS[m],
                                        in1=wdense, op0=_ALU.mult, op1=_ALU.add)
                            nc.vector.affine_mul_reduce(
                                out=wgd, accum_out=gsum_d[:, j:j + 1],
                                in0=wdense, in1=pt[:, g, :], scale=1.0, bias=0.0)
            else:
                nc.vector.memset(sums, 1.0)

            # lse, weighted gather sums, per-core partial
            lse = small.tile([P, TILES], mybir.dt.float32)
            nc.scalar.activation(out=lse, in_=sums, func=_ACT.Ln)
            wg = small.tile([P, TILES, K], mybir.dt.float32)
            gsum = small.tile([P, TILES], mybir.dt.float32)
            if gath_tiles is not None:
                for j in range(TILES):
                    nc.vector.affine_mul_reduce(
                        out=wg[:, j, :], accum_out=gsum[:, j:j + 1],
                        in0=w[:, j, :], in1=gath_tiles[j], scale=1.0, bias=0.0)
            else:
                nc.vector.tensor_mul(out=wg, in0=w, in1=gath)
                nc.vector.reduce_sum(out=gsum, in_=wg, axis=mybir.AxisListType.X)
            if dve_set:
                nc.vector.tensor_add(out=gsum, in0=gsum, in1=gsum_d)
            loss = small.tile([P, TILES], mybir.dt.float32)
            nc.vector.tensor_mul(out=loss, in0=wsum, in1=lse)
            nc.vector.tensor_sub(out=loss, in0=loss, in1=gsum)
            part = small.tile([P, 1], mybir.dt.float32)
            nc.vector.reduce_sum(out=part, in_=loss, axis=mybir.AxisListType.X)
            nc.sync.dma_start(out=out.ap(), in_=part)
    nc.compile()
    return nc


def _get_nc():
    global _NC
    if _NC is None:
        _NC = _build()
    return _NC


def _shard_inputs(pred, target):
    bpc = B // CORES
    in_maps = []
    for c in range(CORES):
        in_maps.append({
            "pred": np.ascontiguousarray(
                pred[c * bpc:(c + 1) * bpc].reshape(SHARD, C), dtype=np.float32),
            "target": np.ascontiguousarray(
                target[c * bpc:(c + 1) * bpc].reshape(SHARD), dtype=np.int32),
        })
    return in_maps


def _run(pred, target, **kwargs):
    nc = _get_nc()
    return bass_utils.run_bass_kernel_spmd(
        nc, _shard_inputs(pred, target), core_ids=list(range(CORES)), **kwargs)


def kernel(pred, target):
    res = _run(pred, target)
    total = sum(float(r["partial"].astype(np.float64).sum()) for r in res.results)
    return np.asarray(total / (B * T), dtype=np.float32)
